# revision 1
# baseline (speedup 1.0000x reference)
"""AttentionBlock (GroupNorm + single-head self-attention + residual) on 8 TRN2 cores.

Data-parallel over batch: 32 samples -> 4 per core; weights replicated.

Per-sample dataflow (all layouts native, no on-device transposes):
  x   [C=512, N=1024]  (4 sbuf tiles of [128, 1024], partition = channel)
  GroupNorm stats: per-partition bn_stats -> tiny matmul with group-membership
    matrix (cross-partition reduce) -> rstd via exp(-0.5*ln(var+eps)) on ACT
    -> tiny matmul broadcast back to partitions -> per-partition affine.
  q,k [D, N] = WqT-slices.T @ xn       (lhsT = host-transposed weights)
  vT  [N, D] = xn-slices.T @ WvT
  eT  [M, N] = exp(scale * k-slices.T @ q)   (softmax numerator, transposed)
  rsum[1, N] = ones.T @ eT  (matmul over partition dim = softmax denominator)
  rb  [128, N] = broadcast of 1/rsum via K=1 matmul + exp(-ln(.)) on ACT
  oT  [D, N] = (vT-slices.T @ eT) * rb       (normalization folded into evict)
  out [E, N] = WpT-slices.T @ oT + (bp + gn_shift) + x*gn_scale   (residual
               recomputed from raw x so it stays exact fp32)

Matmuls run in float32r (inputs written as f32r by the producing DVE/ACT op,
as the BIR verifier requires); fp32 accumulation in PSUM.
"""

import sys

if "/opt/trn_rl_repo" not in sys.path:
    sys.path.insert(0, "/opt/trn_rl_repo")

from contextlib import ExitStack

import numpy as np

import concourse.bass as bass
import concourse.tile as tile
from concourse import bacc, mybir
from concourse.bass_utils import run_bass_kernel_spmd

N_CORES = 8
B, C, H, W = 32, 512, 32, 32
HW = H * W            # tokens per sample (N)
SPC = B // N_CORES    # samples per core
G = 8                 # groups
GSZ = C // G          # channels per group (64)
EPS = 1e-5
P = 128               # partitions
CT = C // P           # channel tiles (4)
NT = HW // P          # token tiles (8)
NCHUNK = HW // 512    # 512-wide free-dim chunks over tokens (2)
SCALE = C ** -0.5

F32 = mybir.dt.float32
F32R = mybir.dt.float32r
AF = mybir.ActivationFunctionType
ALU = mybir.AluOpType


def _declare_io(nc):
    def inp(name, shape):
        return nc.dram_tensor(name, list(shape), F32, kind="ExternalInput").ap()

    aps = {
        "x": inp("x", (SPC, C, HW)),
        "wqt": inp("wqt", (C, C)),
        "wkt": inp("wkt", (C, C)),
        "wvt": inp("wvt", (C, C)),
        "wpt": inp("wpt", (C, C)),
        "bq": inp("bq", (C,)),
        "bk": inp("bk", (C,)),
        "bvb": inp("bvb", (P, C)),
        "bp": inp("bp", (C,)),
        "gamma": inp("gamma", (C,)),
        "beta": inp("beta", (C,)),
        "member": inp("member", (P, 2)),
        "member_t": inp("member_t", (2, P)),
        "ones_col": inp("ones_col", (P, 1)),
        "ones_row": inp("ones_row", (1, P)),
        "out": nc.dram_tensor("out", [SPC, C, HW], F32, kind="ExternalOutput").ap(),
    }
    return aps


def _build_tile_kernel(ctx: ExitStack, tc: tile.TileContext, aps):
    nc = tc.nc

    singles = ctx.enter_context(tc.tile_pool(name="singles", bufs=1))
    wstage = ctx.enter_context(tc.tile_pool(name="wstage", bufs=2))
    wpool = ctx.enter_context(tc.tile_pool(name="wpool", bufs=1))

    # ---- weights: DMA f32 -> DVE round to f32r (verifier-required producer) ----
    wr = {}
    for wname in ("wqt", "wkt", "wvt", "wpt"):
        tiles = []
        for ct in range(CT):
            st = wstage.tile([P, C], F32, tag="wstage")
            nc.sync.dma_start(out=st[:], in_=aps[wname][ct * P:(ct + 1) * P, :])
            t = wpool.tile([P, C], F32R, tag=f"{wname}{ct}")
            nc.vector.tensor_copy(out=t[:], in_=st[:])
            tiles.append(t)
        wr[wname] = tiles

    # ---- small constants ----
    def load_cols(name):
        # [C] dram vector -> [128, CT] sbuf (partition p, col t) = v[t*128 + p]
        t = singles.tile([P, CT], F32, tag=name + "c")
        nc.sync.dma_start(out=t[:], in_=aps[name].rearrange("(t p) -> p t", p=P))
        return t

    bq_c = load_cols("bq")
    bk_c = load_cols("bk")
    bp_c = load_cols("bp")
    gamma_c = load_cols("gamma")
    beta_c = load_cols("beta")

    bvb = singles.tile([P, C], F32, tag="bvb")
    nc.sync.dma_start(out=bvb[:], in_=aps["bvb"][:])

    member = singles.tile([P, 2], F32, tag="member")
    nc.sync.dma_start(out=member[:], in_=aps["member"][:])
    member_t = singles.tile([2, P], F32, tag="member_t")
    nc.sync.dma_start(out=member_t[:], in_=aps["member_t"][:])

    ones_col_f = singles.tile([P, 1], F32, tag="ones_col_f")
    nc.sync.dma_start(out=ones_col_f[:], in_=aps["ones_col"][:])
    ones_col = singles.tile([P, 1], F32R, tag="ones_col")
    nc.vector.tensor_copy(out=ones_col[:], in_=ones_col_f[:])
    ones_row_f = singles.tile([1, P], F32, tag="ones_row_f")
    nc.sync.dma_start(out=ones_row_f[:], in_=aps["ones_row"][:])
    ones_row = singles.tile([1, P], F32R, tag="ones_row")
    nc.vector.tensor_copy(out=ones_row[:], in_=ones_row_f[:])

    eps_t = singles.tile([2, 1], F32, tag="eps_t")
    nc.vector.memset(eps_t[:], EPS)

    # ---- pools for per-sample state ----
    xpool = ctx.enter_context(tc.tile_pool(name="xpool", bufs=1))
    xnpool = ctx.enter_context(tc.tile_pool(name="xnpool", bufs=1))
    qpool = ctx.enter_context(tc.tile_pool(name="qpool", bufs=1))
    kpool = ctx.enter_context(tc.tile_pool(name="kpool", bufs=1))
    vpool = ctx.enter_context(tc.tile_pool(name="vpool", bufs=1))
    epool = ctx.enter_context(tc.tile_pool(name="epool", bufs=1))
    opool = ctx.enter_context(tc.tile_pool(name="opool", bufs=1))
    stat = ctx.enter_context(tc.tile_pool(name="stat", bufs=2))
    rpool = ctx.enter_context(tc.tile_pool(name="rpool", bufs=2))
    respool = ctx.enter_context(tc.tile_pool(name="respool", bufs=3))

    psum_big = ctx.enter_context(tc.tile_pool(name="psum_big", bufs=3, space="PSUM"))
    psum_sm = ctx.enter_context(tc.tile_pool(name="psum_sm", bufs=2, space="PSUM"))

    for s in range(SPC):
        # ================= load x =================
        xr = []
        for ct in range(CT):
            t = xpool.tile([P, HW], F32, tag=f"x{ct}")
            nc.sync.dma_start(out=t[:], in_=aps["x"][s, ct * P:(ct + 1) * P, :])
            xr.append(t)

        # ================= GroupNorm stats =================
        # partials[p, 2t] = mean_p(tile t), partials[p, 2t+1] = E_p[x^2](tile t)
        partials = stat.tile([P, CT, 2], F32, tag="partials")
        for ct in range(CT):
            st6 = stat.tile([P, 2, 6], F32, tag="st6")
            nc.vector.bn_stats(out=st6[:, 0, :], in_=xr[ct][:, 0:512])
            nc.vector.bn_stats(out=st6[:, 1, :], in_=xr[ct][:, 512:1024])
            nc.vector.bn_aggr(out=partials[:, ct, :], in_=st6[:])
            # var -> E[x^2] = var + mean^2
            nc.vector.scalar_tensor_tensor(
                out=partials[:, ct, 1:2], in0=partials[:, ct, 0:1],
                scalar=partials[:, ct, 0:1], in1=partials[:, ct, 1:2],
                op0=ALU.mult, op1=ALU.add)

        # group-reduce across partitions: stats[j, 2t+k] (member is 1/64-valued)
        st_ps = psum_sm.tile([2, CT * 2], F32, tag="sm")
        nc.tensor.matmul(st_ps[:], member[:], partials[:].rearrange("p t j -> p (t j)"),
                         start=True, stop=True)
        stats = stat.tile([2, CT, 2], F32, tag="stats")
        nc.vector.tensor_copy(out=stats[:], in_=st_ps[:].rearrange("p (t j) -> p t j", j=2))
        mv = stats[:, :, 0]   # [2, CT] group means
        sv = stats[:, :, 1]   # [2, CT] group E[x^2]
        # var = E[x^2] - mean^2
        msq = stat.tile([2, CT], F32, tag="msq")
        nc.vector.tensor_mul(out=msq[:], in0=mv, in1=mv)
        nc.vector.tensor_sub(out=sv, in0=sv, in1=msq[:])
        # rstd = exp(-0.5 * ln(var + eps))   (Ln+Exp share one ACT table set)
        nc.scalar.activation(out=sv, in_=sv, func=AF.Ln, bias=eps_t[:], scale=1.0)
        nc.scalar.activation(out=sv, in_=sv, func=AF.Exp, scale=-0.5)
        # ab[j, 2t] = rstd, ab[j, 2t+1] = -mean*rstd
        ab = stat.tile([2, CT, 2], F32, tag="ab")
        nc.vector.tensor_copy(out=ab[:, :, 0], in_=sv)
        nc.vector.scalar_tensor_tensor(out=ab[:, :, 1], in0=mv, scalar=-1.0,
                                       in1=sv, op0=ALU.mult, op1=ALU.mult)
        # broadcast to partitions: sb[p, 2t+k] = ab[glocal(p), 2t+k]
        sb_ps = psum_sm.tile([P, CT * 2], F32, tag="sm")
        nc.tensor.matmul(sb_ps[:], member_t[:], ab[:].rearrange("p t j -> p (t j)"),
                         start=True, stop=True)
        sb = stat.tile([P, CT, 2], F32, tag="sb")
        nc.vector.tensor_copy(out=sb[:], in_=sb_ps[:].rearrange("p (t j) -> p t j", j=2))

        # per-channel affine: scale = gamma*rstd ; shift = gamma*(-mean*rstd)+beta
        sc = stat.tile([P, CT], F32, tag="sc")
        sh = stat.tile([P, CT], F32, tag="sh")
        bpt = stat.tile([P, CT], F32, tag="bpt")
        xn = []
        for ct in range(CT):
            nc.vector.tensor_scalar_mul(out=sc[:, ct:ct + 1], in0=gamma_c[:, ct:ct + 1],
                                        scalar1=sb[:, ct, 0:1])
            nc.vector.scalar_tensor_tensor(out=sh[:, ct:ct + 1], in0=gamma_c[:, ct:ct + 1],
                                           scalar=sb[:, ct, 1:2], in1=beta_c[:, ct:ct + 1],
                                           op0=ALU.mult, op1=ALU.add)
            # fold GN shift into projection bias for the epilogue
            nc.vector.tensor_add(out=bpt[:, ct:ct + 1], in0=bp_c[:, ct:ct + 1],
                                 in1=sh[:, ct:ct + 1])
            t = xnpool.tile([P, HW], F32R, tag=f"xn{ct}")
            nc.vector.tensor_scalar(out=t[:], in0=xr[ct][:],
                                    scalar1=sc[:, ct:ct + 1], scalar2=sh[:, ct:ct + 1],
                                    op0=ALU.mult, op1=ALU.add)
            xn.append(t)

        # ================= Q, K projections: q[d, n] =================
        qk = {}
        for pname, wt, bcol in (("q", wr["wqt"], bq_c), ("k", wr["wkt"], bk_c)):
            tiles = []
            for dt in range(CT):
                t = (qpool if pname == "q" else kpool).tile([P, HW], F32R, tag=f"{pname}{dt}")
                for jc in range(NCHUNK):
                    ps = psum_big.tile([P, 512], F32, tag="big")
                    for ct in range(CT):
                        nc.tensor.matmul(ps[:], wt[ct][:, dt * P:(dt + 1) * P],
                                         xn[ct][:, jc * 512:(jc + 1) * 512],
                                         start=(ct == 0), stop=(ct == CT - 1))
                    nc.vector.tensor_scalar_add(out=t[:, jc * 512:(jc + 1) * 512],
                                                in0=ps[:], scalar1=bcol[:, dt:dt + 1])
                tiles.append(t)
            qk[pname] = tiles
        q, k = qk["q"], qk["k"]

        # ================= V projection, transposed: vT[n, d] =================
        vT = []
        for nt in range(NT):
            t = vpool.tile([P, C], F32R, tag=f"v{nt}")
            ps = psum_big.tile([P, 512], F32, tag="big")
            for ct in range(CT):
                nc.tensor.matmul(ps[:], xn[ct][:, nt * P:(nt + 1) * P], wr["wvt"][ct][:],
                                 start=(ct == 0), stop=(ct == CT - 1))
            nc.vector.tensor_add(out=t[:], in0=ps[:], in1=bvb[:])
            vT.append(t)

        # ================= scores^T + exp: eT[m, n] =================
        eT = []
        for mt in range(NT):
            t = epool.tile([P, HW], F32R, tag=f"e{mt}")
            for jc in range(NCHUNK):
                ps = psum_big.tile([P, 512], F32, tag="big")
                for dt in range(CT):
                    nc.tensor.matmul(ps[:], k[dt][:, mt * P:(mt + 1) * P],
                                     q[dt][:, jc * 512:(jc + 1) * 512],
                                     start=(dt == 0), stop=(dt == CT - 1))
                nc.scalar.activation(out=t[:, jc * 512:(jc + 1) * 512], in_=ps[:],
                                     func=AF.Exp, scale=SCALE)
            eT.append(t)

        # ================= softmax denominator -> rb = 1/rowsum bcast =========
        rs_ps = psum_sm.tile([1, HW], F32, tag="sm")
        for jc in range(NCHUNK):
            for mt in range(NT):
                nc.tensor.matmul(rs_ps[:, jc * 512:(jc + 1) * 512], ones_col[:],
                                 eT[mt][:, jc * 512:(jc + 1) * 512],
                                 start=(mt == 0), stop=(mt == NT - 1))
        rs_sb = rpool.tile([1, HW], F32R, tag="rs")
        nc.vector.tensor_copy(out=rs_sb[:], in_=rs_ps[:])
        rb = rpool.tile([P, HW], F32, tag="rb")
        for jc in range(NCHUNK):
            ps = psum_sm.tile([P, 512], F32, tag="sm")
            nc.tensor.matmul(ps[:], ones_row[:], rs_sb[:, jc * 512:(jc + 1) * 512],
                             start=True, stop=True)
            # 1/x = exp(-ln(x)), both in the already-loaded ACT table set
            nc.scalar.activation(out=rb[:, jc * 512:(jc + 1) * 512], in_=ps[:],
                                 func=AF.Ln, scale=1.0)
            nc.scalar.activation(out=rb[:, jc * 512:(jc + 1) * 512],
                                 in_=rb[:, jc * 512:(jc + 1) * 512],
                                 func=AF.Exp, scale=-1.0)

        # ================= attn @ V, transposed: oT[d, n] =================
        oT = []
        for dt in range(CT):
            t = opool.tile([P, HW], F32R, tag=f"o{dt}")
            for jc in range(NCHUNK):
                ps = psum_big.tile([P, 512], F32, tag="big")
                for mt in range(NT):
                    nc.tensor.matmul(ps[:], vT[mt][:, dt * P:(dt + 1) * P],
                                     eT[mt][:, jc * 512:(jc + 1) * 512],
                                     start=(mt == 0), stop=(mt == NT - 1))
                # softmax normalization folded into the eviction
                nc.vector.tensor_mul(out=t[:, jc * 512:(jc + 1) * 512], in0=ps[:],
                                     in1=rb[:, jc * 512:(jc + 1) * 512])
            oT.append(t)

        # ================= projection + bias + residual =================
        for et in range(CT):
            for jc in range(NCHUNK):
                ps = psum_big.tile([P, 512], F32, tag="big")
                for dt in range(CT):
                    nc.tensor.matmul(ps[:], wr["wpt"][dt][:, et * P:(et + 1) * P],
                                     oT[dt][:, jc * 512:(jc + 1) * 512],
                                     start=(dt == 0), stop=(dt == CT - 1))
                tmp = respool.tile([P, 512], F32, tag="tmp")
                nc.scalar.activation(out=tmp[:], in_=ps[:], func=AF.Identity,
                                     bias=bpt[:, et:et + 1], scale=1.0)
                res = respool.tile([P, 512], F32, tag="res")
                # out = x*gn_scale + (proj + bp + gn_shift)  (exact-f32 residual)
                nc.vector.scalar_tensor_tensor(
                    out=res[:], in0=xr[et][:, jc * 512:(jc + 1) * 512],
                    scalar=sc[:, et:et + 1], in1=tmp[:], op0=ALU.mult, op1=ALU.add)
                nc.sync.dma_start(
                    out=aps["out"][s, et * P:(et + 1) * P, jc * 512:(jc + 1) * 512],
                    in_=res[:])


def build():
    nc = bacc.Bacc("TRN2", target_bir_lowering=False, debug=False)
    aps = _declare_io(nc)
    with tile.TileContext(nc) as tc:
        with ExitStack() as ctx:
            _build_tile_kernel(ctx, tc, aps)
    nc.compile()
    return nc


_cached_nc = None


def _get_nc():
    global _cached_nc
    if _cached_nc is None:
        _cached_nc = build()
    return _cached_nc


def _host_inputs(gamma, beta, Wq, bq, Wk, bk, Wv, bv, Wp, bp):
    f = lambda a: np.ascontiguousarray(np.asarray(a, dtype=np.float32))
    member = np.zeros((P, 2), np.float32)
    member[:GSZ, 0] = 1.0 / GSZ
    member[GSZ:, 1] = 1.0 / GSZ
    member_t = np.zeros((2, P), np.float32)
    member_t[0, :GSZ] = 1.0
    member_t[1, GSZ:] = 1.0
    return {
        "wqt": f(np.asarray(Wq).T),
        "wkt": f(np.asarray(Wk).T),
        "wvt": f(np.asarray(Wv).T),
        "wpt": f(np.asarray(Wp).T),
        "bq": f(bq), "bk": f(bk), "bp": f(bp),
        "bvb": np.tile(f(bv)[None, :], (P, 1)),
        "gamma": f(gamma), "beta": f(beta),
        "member": member, "member_t": member_t,
        "ones_col": np.ones((P, 1), np.float32),
        "ones_row": np.ones((1, P), np.float32),
    }


def run(inputs, trace=False, **kw):
    """Returns (out [B,C,H,W], BassKernelResults)."""
    nc = _get_nc()
    x = np.ascontiguousarray(np.asarray(inputs["x"], np.float32)).reshape(B, C, HW)
    common = _host_inputs(**{k: v for k, v in inputs.items() if k != "x"})
    in_maps = [dict(common, x=x[c * SPC:(c + 1) * SPC]) for c in range(N_CORES)]
    res = run_bass_kernel_spmd(nc, in_maps, core_ids=list(range(N_CORES)),
                               trace=trace, **kw)
    out = np.concatenate([res.results[c]["out"] for c in range(N_CORES)], axis=0)
    return out.reshape(B, C, H, W), res


def kernel(**inputs):
    out, _ = run(inputs)
    return out
